# revision 21
# baseline (speedup 1.0000x reference)
"""DistinctionLoss Trainium2 kernel (raw bacc, hand-scheduled).

Math (per batch b):
  f_n = x_n / ||x_n||                       (row-normalized features)
  s   = sum_n f_n                           ([D] weighted row sum)
  mean(gram) = ||s||^2 / N^2                (the N x N gram is never built)
  dot_n = f_n . s = rn_n * (x_n . s)
  sim_n = (dot_n - 1)/(N-1);  t_n = 1 - relu(sim_n)
  bce  = -mean(t*log(sc) + (1-t)*log1p(-sc))   (logs clamped at -100)
  loss = bce + 1 - mean_b(||s_b||^2)/N^2

Sharding: data-parallel over B=8 across 8 NeuronCores (1 batch per core).
Features are cast to bf16 on the host (halves DMA, enables DVE 2x mode;
~1e-7 relative error on the ~2.0 loss). Each core returns out[128, 2]:
col 0 = per-partition BCE partial sums, out[0,1] = ||s||^2; the host does
the final (tiny) reduction.

Engine schedule per core (no Tile framework — manual semaphores):
  sync : 4 X-chunk DMAs, final out DMA
  gp   : 3 X-chunk DMAs + scores DMA (parallel SWDGE queue)
  ACT  : table warmups, per-chunk Square, per-chunk rn=Sqrt(1/ssq)->bf16,
         Ln(ls/l1), s copies (PSUM->SBUF), ||s||^2 accum, 7 phase-E
         accum-reduce groups
  DVE  : per-chunk sumsq reduce + reciprocal, score clamps/w/ls_sum,
         phase-E mul + bf16 fold-tree reduce (25 groups) + BCE tail
  PE   : 32 accumulating matmuls (s = sum rn_n x_n), s broadcast matmul
"""

import numpy as np
import ml_dtypes

B = 8
N, D, P = 4096, 256, 128
G = N // P
CHUNKS = [2, 3, 4, 5, 6, 6, 4, 2]
NCH = len(CHUNKS)
OFFS = [sum(CHUNKS[:i]) for i in range(NCH)]
GA = 22
NINV = 1.0 / (N - 1)
LOG_CLAMP = -100.0

_cache = {}


def _build_nc():
    import concourse.bacc as bacc
    import concourse.bass as bass
    from concourse import mybir
    from contextlib import ExitStack

    fp32 = mybir.dt.float32
    bf16 = mybir.dt.bfloat16
    AF = mybir.ActivationFunctionType
    ALU = mybir.AluOpType
    AX = mybir.AxisListType

    nc = bacc.Bacc(
        "TRN2", target_bir_lowering=False, debug=False,
        enable_asserts=False, num_devices=8,
    )

    xbf = nc.dram_tensor("xbf", [N, D], bf16, kind="ExternalInput")
    scores = nc.dram_tensor("scores", [N, 1], fp32, kind="ExternalInput")
    out_d = nc.dram_tensor("out", [1, 2], fp32, kind="ExternalOutput")

    x_r = xbf[:].rearrange("(p g) d -> p g d", p=P)
    sc_r = scores[:].rearrange("(p g) o -> p (g o)", p=P)

    sb = nc.alloc_sbuf_tensor
    x_t = sb("x", [P, G, D], bf16)
    sq_t = [sb(f"sq{i}", [P, CHUNKS[i], D], bf16) for i in range(NCH)]
    ssq_t = sb("ssq", [P, G], fp32)
    issq_t = sb("issq", [P, G], fp32)
    rnbf_t = sb("rnbf", [P, G], bf16)
    sc_t = sb("sc", [P, G], fp32)
    ls_t = sb("ls", [P, G], fp32)
    l1_t = sb("l1", [P, G], fp32)
    w_t = sb("w", [P, G], fp32)
    lssum_t = sb("lssum", [P, 1], fp32)
    pt_t = sb("pt", [P, G, D], bf16)
    f1s_t = sb("f1s", [P, 6, 128], bf16)
    f1_t = sb("f1", [P, GA, 128], bf16)
    f2_t = sb("f2", [P, GA, 64], bf16)
    f3_t = sb("f3", [P, GA, 32], bf16)
    actscr_t = sb("actscr", [P, G - GA, D], fp32)
    draw_t = sb("draw", [P, G], fp32)
    dots_t = sb("dots", [P, G], fp32)
    sim_t = sb("sim", [P, G], fp32)
    rterm_t = sb("rterm", [P, G], fp32)
    rwsum_t = sb("rwsum", [P, 1], fp32)
    onesb_t = sb("onesb", [1, P], bf16)
    onesf_t = sb("onesf", [P, 1], fp32)
    sbf1_t = sb("sbf1", [1, D], bf16)
    sbc_t = sb("sbc", [P, D], bf16)
    sscr_t = sb("sscr", [1, D], fp32)
    warm_t = sb("warm", [1, 3], fp32)
    outfin_t = sb("outfin", [1, 2], fp32)
    outsb_t = sb("outsb", [P, 2], fp32)

    ctx = ExitStack()
    ps_s = ctx.enter_context(nc.psum_tensor([1, D], fp32))
    ps_bc = ctx.enter_context(nc.psum_tensor([P, D], fp32))
    ps_tot = ctx.enter_context(nc.psum_tensor([1, 2], fp32))
    names = (["S_dsc"] + [f"S_dx{k}" for k in range(NCH)] +
             ["S_ln", "S_sq", "S_issq", "S_rnbf", "S_pe", "S_sbf",
              "S_pebc", "S_sbc", "S_mulE", "S_accE", "S_dveE", "S_out", "S_pef", "S_fin", "S_ones", "S_mulG", "S_od"])
    S = {n: ctx.enter_context(nc.semaphore(n)) for n in names}
    S_dx = [S[f"S_dx{k}"] for k in range(NCH)]

    def xsl(k):
        return slice(OFFS[k], OFFS[k] + CHUNKS[k])

    _ob = onesb_t[:]
    sbc_warm_ap = bass.AP(tensor=_ob.tensor, offset=_ob.offset,
                          ap=[_ob.ap[0], [0, 2], _ob.ap[1]])
    _sb = sbc_t[:]
    s_b3g = bass.AP(tensor=_sb.tensor, offset=_sb.offset,
                    ap=[_sb.ap[0], [0, G - GA], _sb.ap[1]])

    with ctx, nc.Block() as block:
        @block.sync
        def _(sync):
            for k in (0, 2, 4, 6):
                sync.dma_start(out=x_t[:, xsl(k), :], in_=x_r[:, xsl(k), :]
                               ).then_inc(S_dx[k], 16)
            sync.wait_ge(S["S_fin"], 1)
            sync.dma_start(out=out_d[:], in_=outfin_t[:]).then_inc(S["S_od"], 16)
            sync.wait_ge(S["S_od"], 16)

        @block.gpsimd
        def _(gp):
            gp.dma_start(out=sc_t[:], in_=sc_r).then_inc(S["S_dsc"], 16)
            for k in (1, 3, 5, 7):
                gp.dma_start(out=x_t[:, xsl(k), :], in_=x_r[:, xsl(k), :]
                             ).then_inc(S_dx[k], 16)

        @block.scalar
        def _(act):
            # front-load the Square/Sqrt ACT tables
            act.activation(out=warm_t[:, 0:1],
                           in_=nc.const_aps.tensor(1.0, (1, 1)), func=AF.Square)
            for k in range(NCH):
                act.wait_ge(S_dx[k], 16)
                act.activation(out=sq_t[k][:], in_=x_t[:, xsl(k), :],
                               func=AF.Square).then_inc(S["S_sq"], 1)
                if k == 0:
                    act.sqrt(warm_t[:, 1:2], nc.const_aps.tensor(1.0, (1, 1)))

                if k >= 1:
                    j = k - 1
                    act.wait_ge(S["S_issq"], j + 1)
                    act.sqrt(rnbf_t[:, xsl(j)], issq_t[:, xsl(j)]
                             ).then_inc(S["S_rnbf"], 1)
            j = NCH - 1
            act.wait_ge(S["S_issq"], j + 1)
            act.sqrt(rnbf_t[:, xsl(j)], issq_t[:, xsl(j)]).then_inc(S["S_rnbf"], 1)
            # s: PSUM -> SBUF bf16; ||s||^2 -> outsb[0,1]
            act.wait_ge(S["S_pe"], 1)
            act.copy(sbf1_t[:], ps_s[:]).then_inc(S["S_sbf"], 1)
            act.activation(
                out=sscr_t[:], in_=ps_s[:], func=AF.Square,
                accum_out=outsb_t[0:1, 1:2],
            ).then_inc(S["S_out"], 1)
            act.wait_ge(S["S_pebc"], 1)
            act.copy(sbc_t[:], ps_bc[:]).then_inc(S["S_sbc"], 1)
            # scores side: Ln table + both Lns hide under the phase-E mul
            act.wait_ge(S["S_dsc"], 16)
            act.activation(out=ls_t[:], in_=sc_t[:], func=AF.Ln)
            act.activation(
                out=l1_t[:], in_=sc_t[:], func=AF.Ln, scale=-1.0, bias=1.0,
            ).then_inc(S["S_ln"], 1)
            # phase-E accum-reduce for groups GA..G-1
            act.wait_ge(S["S_mulE"], 1)
            for g in range(GA, G):
                act.activation(
                    out=actscr_t[:, g - GA, :], in_=pt_t[:, g, :], func=AF.Copy,
                    accum_out=draw_t[:, g:g + 1],
                ).then_inc(S["S_accE"], 1)
            act.wait_ge(S["S_pef"], 1)
            act.copy(outfin_t[:], ps_tot[:]).then_inc(S["S_fin"], 1)

        @block.vector
        def _(dve):
            dve.memset(onesb_t[:], 1.0)
            dve.memset(onesf_t[:], 1.0).then_inc(S["S_ones"], 1)
            dve.memset(outsb_t[:], 0.0)
            for k in range(NCH):
                dve.wait_ge(S["S_sq"], k + 1)
                if CHUNKS[k] >= 4:
                    dve.tensor_add(f1s_t[:, 0:CHUNKS[k], :],
                                   sq_t[k][:, :, 0:128], sq_t[k][:, :, 128:256])
                    dve.drain()
                    dve.tensor_reduce(out=ssq_t[:, xsl(k)],
                                      in_=f1s_t[:, 0:CHUNKS[k], :],
                                      axis=AX.X, op=ALU.add)
                else:
                    dve.tensor_reduce(out=ssq_t[:, xsl(k)], in_=sq_t[k][:],
                                      axis=AX.X, op=ALU.add)
                dve.drain()
                dve.reciprocal(issq_t[:, xsl(k)], ssq_t[:, xsl(k)]
                               ).then_inc(S["S_issq"], 1)
            dve.wait_ge(S["S_sbc"], 1)
            sap = sbc_t[:]
            s_b3 = bass.AP(
                tensor=sap.tensor, offset=sap.offset,
                ap=[sap.ap[0], [0, GA], sap.ap[1]],
            )
            dve.tensor_mul(pt_t[:, GA:G, :], x_t[:, GA:G, :], s_b3g
                           ).then_inc(S["S_mulE"], 1)
            dve.tensor_mul(pt_t[:, 0:GA, :], x_t[:, 0:GA, :], s_b3)
            dve.drain()
            dve.tensor_add(f1_t[:], pt_t[:, 0:GA, 0:128], pt_t[:, 0:GA, 128:256])
            dve.drain()
            dve.tensor_add(f2_t[:], f1_t[:, :, 0:64], f1_t[:, :, 64:128])
            dve.drain()
            dve.tensor_add(f3_t[:], f2_t[:, :, 0:32], f2_t[:, :, 32:64])
            dve.drain()
            dve.tensor_reduce(out=draw_t[:, 0:GA], in_=f3_t[:], axis=AX.X,
                              op=ALU.add)
            dve.drain()
            dve.wait_ge(S["S_ln"], 1)
            # ln1p(-s) >= -16.7 for f32 s < 1, so only ls needs the -100 clamp
            dve.tensor_scalar_max(ls_t[:], ls_t[:], LOG_CLAMP)
            dve.drain()
            dve.tensor_sub(w_t[:], ls_t[:], l1_t[:])
            dve.tensor_reduce(out=lssum_t[:], in_=ls_t[:], axis=AX.X, op=ALU.add)
            dve.drain()
            dve.wait_ge(S["S_accE"], G - GA)
            dve.tensor_mul(dots_t[:], draw_t[:], rnbf_t[:])
            dve.drain()
            dve.tensor_scalar(
                out=sim_t[:], in0=dots_t[:], scalar1=1.0, scalar2=NINV,
                op0=ALU.subtract, op1=ALU.mult,
            )
            dve.drain()
            dve.scalar_tensor_tensor(
                out=rterm_t[:], in0=sim_t[:], scalar=0.0, in1=w_t[:],
                op0=ALU.max, op1=ALU.mult, accum_out=rwsum_t[:],
            )
            dve.drain()
            dve.tensor_sub(outsb_t[:, 0:1], lssum_t[:], rwsum_t[:]
                           ).then_inc(S["S_dveE"], 1)

        @block.tensor
        def _(pe):
            # HAM warmup: ~4us of dummy matmuls so real ones run at 8/8 rate
            pe.wait_ge(S["S_ones"], 1)
            for _ in range(18):
                pe.matmul(ps_bc[:, 0:D], onesb_t[:], sbc_warm_ap,
                          start=True, stop=True)
            mm = None
            for k in range(NCH):
                pe.wait_ge(S["S_rnbf"], k + 1)
                for gl in range(CHUNKS[k]):
                    g = OFFS[k] + gl
                    mm = pe.matmul(
                        ps_s[:], rnbf_t[:, g:g + 1], x_t[:, g, :],
                        start=(g == 0), stop=(g == G - 1),
                    )
            mm.then_inc(S["S_pe"], 1)
            pe.wait_ge(S["S_sbf"], 1)
            pe.matmul(ps_bc[:], onesb_t[:], sbf1_t[:], start=True, stop=True
                      ).then_inc(S["S_pebc"], 1)
            pe.wait_ge(S["S_dveE"], 1)
            pe.wait_ge(S["S_out"], 1)
            pe.matmul(ps_tot[:], onesf_t[:], outsb_t[:], start=True, stop=True
                      ).then_inc(S["S_pef"], 1)

    nc.finalize()
    return nc


def _get_nc():
    if "nc" not in _cache:
        _cache["nc"] = _build_nc()
    return _cache["nc"]


def run_on_device(features: np.ndarray, scores: np.ndarray, trace: bool = False,
                  tmpdir: str | None = None):
    """Returns (per_core_outputs [8, 128, 2] float64, BassKernelResults)."""
    from concourse.bass_utils import run_bass_kernel_spmd

    nc = _get_nc()
    in_maps = []
    for c in range(B):
        in_maps.append({
            "xbf": np.ascontiguousarray(features[c]).astype(ml_dtypes.bfloat16),
            "scores": np.ascontiguousarray(scores[c]).astype(np.float32),
        })
    res = run_bass_kernel_spmd(nc, in_maps, core_ids=list(range(B)),
                               trace=trace, tmpdir=tmpdir)
    outs = np.stack([res.results[c]["out"].reshape(2) for c in range(B)])
    return outs.astype(np.float64), res


def kernel(features: np.ndarray, scores: np.ndarray) -> np.ndarray:
    outs, _ = run_on_device(features, scores)
    bce_sums = outs[:, 0]                         # per-batch BCE sums
    ssqs = outs[:, 1]                             # per-batch ||s||^2
    bce = np.mean(-bce_sums / N)
    feat = 1.0 - np.sum(ssqs) / (B * float(N) * float(N))
    return np.asarray(bce + feat, dtype=np.float32)


# revision 22
# speedup vs baseline: 1.0208x; 1.0208x over previous
"""DistinctionLoss Trainium2 kernel (raw bacc, hand-scheduled).

Math (per batch b):
  f_n = x_n / ||x_n||                       (row-normalized features)
  s   = sum_n f_n                           ([D] weighted row sum)
  mean(gram) = ||s||^2 / N^2                (the N x N gram is never built)
  dot_n = f_n . s = rn_n * (x_n . s)
  sim_n = (dot_n - 1)/(N-1);  t_n = 1 - relu(sim_n)
  bce  = -mean(t*log(sc) + (1-t)*log1p(-sc))   (logs clamped at -100)
  loss = bce + 1 - mean_b(||s_b||^2)/N^2

Sharding: data-parallel over B=8 across 8 NeuronCores (1 batch per core).
Features are cast to bf16 on the host (halves DMA, enables DVE 2x mode;
~1e-7 relative error on the ~2.0 loss). Each core returns out[128, 2]:
col 0 = per-partition BCE partial sums, out[0,1] = ||s||^2; the host does
the final (tiny) reduction.

Engine schedule per core (no Tile framework — manual semaphores):
  sync : 4 X-chunk DMAs, final out DMA
  gp   : 3 X-chunk DMAs + scores DMA (parallel SWDGE queue)
  ACT  : table warmups, per-chunk Square, per-chunk rn=Sqrt(1/ssq)->bf16,
         Ln(ls/l1), s copies (PSUM->SBUF), ||s||^2 accum, 7 phase-E
         accum-reduce groups
  DVE  : per-chunk sumsq reduce + reciprocal, score clamps/w/ls_sum,
         phase-E mul + bf16 fold-tree reduce (25 groups) + BCE tail
  PE   : 32 accumulating matmuls (s = sum rn_n x_n), s broadcast matmul
"""

import numpy as np
import ml_dtypes

B = 8
N, D, P = 4096, 256, 128
G = N // P
CHUNKS = [2, 3, 4, 5, 6, 6, 4, 2]
NCH = len(CHUNKS)
OFFS = [sum(CHUNKS[:i]) for i in range(NCH)]
GA = 22
NINV = 1.0 / (N - 1)
LOG_CLAMP = -100.0

_cache = {}


def _build_nc():
    import concourse.bacc as bacc
    import concourse.bass as bass
    from concourse import mybir
    from contextlib import ExitStack

    fp32 = mybir.dt.float32
    bf16 = mybir.dt.bfloat16
    AF = mybir.ActivationFunctionType
    ALU = mybir.AluOpType
    AX = mybir.AxisListType

    nc = bacc.Bacc(
        "TRN2", target_bir_lowering=False, debug=False,
        enable_asserts=False, num_devices=8,
    )

    xbf = nc.dram_tensor("xbf", [N, D], bf16, kind="ExternalInput")
    scores = nc.dram_tensor("scores", [N, 1], fp32, kind="ExternalInput")
    out_d = nc.dram_tensor("out", [1, 2], fp32, kind="ExternalOutput")

    x_r = xbf[:].rearrange("(p g) d -> p g d", p=P)
    sc_r = scores[:].rearrange("(p g) o -> p (g o)", p=P)

    sb = nc.alloc_sbuf_tensor
    x_t = sb("x", [P, G, D], bf16)
    sq_t = [sb(f"sq{i}", [P, CHUNKS[i], D], bf16) for i in range(NCH)]
    ssq_t = sb("ssq", [P, G], fp32)
    issq_t = sb("issq", [P, G], fp32)
    rnbf_t = sb("rnbf", [P, G], bf16)
    sc_t = sb("sc", [P, G], fp32)
    ls_t = sb("ls", [P, G], fp32)
    l1_t = sb("l1", [P, G], fp32)
    w_t = sb("w", [P, G], fp32)
    lssum_t = sb("lssum", [P, 1], fp32)
    pt_t = sb("pt", [P, G, D], bf16)
    f1s_t = sb("f1s", [P, 6, 128], bf16)
    f1_t = sb("f1", [P, GA, 128], bf16)
    f2_t = sb("f2", [P, GA, 64], bf16)
    f3_t = sb("f3", [P, GA, 32], bf16)
    actscr_t = sb("actscr", [P, G - GA, D], fp32)
    draw_t = sb("draw", [P, G], fp32)
    dots_t = sb("dots", [P, G], fp32)
    sim_t = sb("sim", [P, G], fp32)
    rterm_t = sb("rterm", [P, G], fp32)
    rwsum_t = sb("rwsum", [P, 1], fp32)
    onesb_t = sb("onesb", [1, P], bf16)
    onesf_t = sb("onesf", [P, 1], fp32)
    sbf1_t = sb("sbf1", [1, D], bf16)
    sbc_t = sb("sbc", [P, D], bf16)
    sscr_t = sb("sscr", [1, D], fp32)
    warm_t = sb("warm", [1, 3], fp32)
    outfin_t = sb("outfin", [1, 2], fp32)
    outsb_t = sb("outsb", [P, 2], fp32)

    ctx = ExitStack()
    ps_s = ctx.enter_context(nc.psum_tensor([1, D], fp32))
    ps_bc = ctx.enter_context(nc.psum_tensor([P, D], fp32))
    ps_tot = ctx.enter_context(nc.psum_tensor([1, 2], fp32))
    names = (["S_dsc"] + [f"S_dx{k}" for k in range(NCH)] +
             ["S_ln", "S_sq", "S_issq", "S_rnbf", "S_pe", "S_sbf",
              "S_pebc", "S_sbc", "S_mulE", "S_accE", "S_dveE", "S_out", "S_pef", "S_fin", "S_ones", "S_mulG", "S_od"])
    S = {n: ctx.enter_context(nc.semaphore(n)) for n in names}
    S_dx = [S[f"S_dx{k}"] for k in range(NCH)]

    def xsl(k):
        return slice(OFFS[k], OFFS[k] + CHUNKS[k])

    _ob = onesb_t[:]
    sbc_warm_ap = bass.AP(tensor=_ob.tensor, offset=_ob.offset,
                          ap=[_ob.ap[0], [0, 2], _ob.ap[1]])
    _sb = sbc_t[:]
    s_b3g = bass.AP(tensor=_sb.tensor, offset=_sb.offset,
                    ap=[_sb.ap[0], [0, G - GA], _sb.ap[1]])

    with ctx, nc.Block() as block:
        @block.sync
        def _(sync):
            for k in (0, 2, 4, 6):
                sync.dma_start(out=x_t[:, xsl(k), :], in_=x_r[:, xsl(k), :]
                               ).then_inc(S_dx[k], 16)
            sync.wait_ge(S["S_fin"], 1)
            sync.dma_start(out=out_d[:], in_=outfin_t[:]).then_inc(S["S_od"], 16)
            sync.wait_ge(S["S_od"], 16)

        @block.gpsimd
        def _(gp):
            gp.dma_start(out=sc_t[:], in_=sc_r).then_inc(S["S_dsc"], 16)
            for k in (1, 3, 5, 7):
                gp.dma_start(out=x_t[:, xsl(k), :], in_=x_r[:, xsl(k), :]
                             ).then_inc(S_dx[k], 16)

        @block.scalar
        def _(act):
            # front-load the Square/Sqrt ACT tables
            act.activation(out=warm_t[:, 0:1],
                           in_=nc.const_aps.tensor(1.0, (1, 1)), func=AF.Square)
            for k in range(NCH):
                act.wait_ge(S_dx[k], 16)
                act.activation(out=sq_t[k][:], in_=x_t[:, xsl(k), :],
                               func=AF.Square).then_inc(S["S_sq"], 1)
                if k == 0:
                    act.sqrt(warm_t[:, 1:2], nc.const_aps.tensor(1.0, (1, 1)))

                if k >= 1:
                    j = k - 1
                    act.wait_ge(S["S_issq"], j + 1)
                    act.sqrt(rnbf_t[:, xsl(j)], issq_t[:, xsl(j)]
                             ).then_inc(S["S_rnbf"], 1)
            j = NCH - 1
            act.wait_ge(S["S_issq"], j + 1)
            act.sqrt(rnbf_t[:, xsl(j)], issq_t[:, xsl(j)]).then_inc(S["S_rnbf"], 1)
            # s: PSUM -> SBUF bf16; ||s||^2 -> outsb[0,1]
            act.wait_ge(S["S_pe"], 1)
            act.copy(sbf1_t[:], ps_s[:]).then_inc(S["S_sbf"], 1)
            act.wait_ge(S["S_pebc"], 1)
            act.copy(sbc_t[:], ps_bc[:]).then_inc(S["S_sbc"], 1)
            act.activation(
                out=sscr_t[:], in_=ps_s[:], func=AF.Square,
                accum_out=outsb_t[0:1, 1:2],
            ).then_inc(S["S_out"], 1)
            # scores side: Ln table + both Lns hide under the phase-E mul
            act.wait_ge(S["S_dsc"], 16)
            act.activation(out=ls_t[:], in_=sc_t[:], func=AF.Ln)
            act.activation(
                out=l1_t[:], in_=sc_t[:], func=AF.Ln, scale=-1.0, bias=1.0,
            ).then_inc(S["S_ln"], 1)
            # phase-E accum-reduce for groups GA..G-1
            act.wait_ge(S["S_mulE"], 1)
            for g in range(GA, G):
                act.activation(
                    out=actscr_t[:, g - GA, :], in_=pt_t[:, g, :], func=AF.Copy,
                    accum_out=draw_t[:, g:g + 1],
                ).then_inc(S["S_accE"], 1)
            act.wait_ge(S["S_pef"], 1)
            act.copy(outfin_t[:], ps_tot[:]).then_inc(S["S_fin"], 1)

        @block.vector
        def _(dve):
            dve.memset(onesb_t[:], 1.0)
            dve.memset(onesf_t[:], 1.0).then_inc(S["S_ones"], 1)
            dve.memset(outsb_t[:], 0.0)
            for k in range(NCH):
                dve.wait_ge(S["S_sq"], k + 1)
                if CHUNKS[k] >= 4:
                    dve.tensor_add(f1s_t[:, 0:CHUNKS[k], :],
                                   sq_t[k][:, :, 0:128], sq_t[k][:, :, 128:256])
                    dve.drain()
                    dve.tensor_reduce(out=ssq_t[:, xsl(k)],
                                      in_=f1s_t[:, 0:CHUNKS[k], :],
                                      axis=AX.X, op=ALU.add)
                else:
                    dve.tensor_reduce(out=ssq_t[:, xsl(k)], in_=sq_t[k][:],
                                      axis=AX.X, op=ALU.add)
                dve.drain()
                dve.reciprocal(issq_t[:, xsl(k)], ssq_t[:, xsl(k)]
                               ).then_inc(S["S_issq"], 1)
            dve.wait_ge(S["S_sbc"], 1)
            sap = sbc_t[:]
            s_b3 = bass.AP(
                tensor=sap.tensor, offset=sap.offset,
                ap=[sap.ap[0], [0, GA], sap.ap[1]],
            )
            dve.tensor_mul(pt_t[:, GA:G, :], x_t[:, GA:G, :], s_b3g
                           ).then_inc(S["S_mulE"], 1)
            dve.tensor_mul(pt_t[:, 0:GA, :], x_t[:, 0:GA, :], s_b3)
            dve.drain()
            dve.tensor_add(f1_t[:], pt_t[:, 0:GA, 0:128], pt_t[:, 0:GA, 128:256])
            dve.drain()
            dve.tensor_add(f2_t[:], f1_t[:, :, 0:64], f1_t[:, :, 64:128])
            dve.drain()
            dve.tensor_add(f3_t[:], f2_t[:, :, 0:32], f2_t[:, :, 32:64])
            dve.drain()
            dve.tensor_reduce(out=draw_t[:, 0:GA], in_=f3_t[:], axis=AX.X,
                              op=ALU.add)
            dve.drain()
            dve.wait_ge(S["S_ln"], 1)
            # ln1p(-s) >= -16.7 for f32 s < 1, so only ls needs the -100 clamp
            dve.tensor_scalar_max(ls_t[:], ls_t[:], LOG_CLAMP)
            dve.drain()
            dve.tensor_sub(w_t[:], ls_t[:], l1_t[:])
            dve.tensor_reduce(out=lssum_t[:], in_=ls_t[:], axis=AX.X, op=ALU.add)
            dve.drain()
            dve.wait_ge(S["S_accE"], G - GA)
            dve.tensor_mul(dots_t[:], draw_t[:], rnbf_t[:])
            dve.drain()
            dve.tensor_scalar(
                out=sim_t[:], in0=dots_t[:], scalar1=1.0, scalar2=NINV,
                op0=ALU.subtract, op1=ALU.mult,
            )
            dve.drain()
            dve.scalar_tensor_tensor(
                out=rterm_t[:], in0=sim_t[:], scalar=0.0, in1=w_t[:],
                op0=ALU.max, op1=ALU.mult, accum_out=rwsum_t[:],
            )
            dve.drain()
            dve.tensor_sub(outsb_t[:, 0:1], lssum_t[:], rwsum_t[:]
                           ).then_inc(S["S_dveE"], 1)

        @block.tensor
        def _(pe):
            # HAM warmup: ~4us of dummy matmuls so real ones run at 8/8 rate
            pe.wait_ge(S["S_ones"], 1)
            for _ in range(18):
                pe.matmul(ps_bc[:, 0:D], onesb_t[:], sbc_warm_ap,
                          start=True, stop=True)
            mm = None
            for k in range(NCH):
                pe.wait_ge(S["S_rnbf"], k + 1)
                for gl in range(CHUNKS[k]):
                    g = OFFS[k] + gl
                    mm = pe.matmul(
                        ps_s[:], rnbf_t[:, g:g + 1], x_t[:, g, :],
                        start=(g == 0), stop=(g == G - 1),
                    )
            mm.then_inc(S["S_pe"], 1)
            pe.wait_ge(S["S_sbf"], 1)
            pe.matmul(ps_bc[:], onesb_t[:], sbf1_t[:], start=True, stop=True
                      ).then_inc(S["S_pebc"], 1)
            pe.wait_ge(S["S_dveE"], 1)
            pe.wait_ge(S["S_out"], 1)
            pe.matmul(ps_tot[:], onesf_t[:], outsb_t[:], start=True, stop=True
                      ).then_inc(S["S_pef"], 1)

    nc.finalize()
    return nc


def _get_nc():
    if "nc" not in _cache:
        _cache["nc"] = _build_nc()
    return _cache["nc"]


def run_on_device(features: np.ndarray, scores: np.ndarray, trace: bool = False,
                  tmpdir: str | None = None):
    """Returns (per_core_outputs [8, 128, 2] float64, BassKernelResults)."""
    from concourse.bass_utils import run_bass_kernel_spmd

    nc = _get_nc()
    in_maps = []
    for c in range(B):
        in_maps.append({
            "xbf": np.ascontiguousarray(features[c]).astype(ml_dtypes.bfloat16),
            "scores": np.ascontiguousarray(scores[c]).astype(np.float32),
        })
    res = run_bass_kernel_spmd(nc, in_maps, core_ids=list(range(B)),
                               trace=trace, tmpdir=tmpdir)
    outs = np.stack([res.results[c]["out"].reshape(2) for c in range(B)])
    return outs.astype(np.float64), res


def kernel(features: np.ndarray, scores: np.ndarray) -> np.ndarray:
    outs, _ = run_on_device(features, scores)
    bce_sums = outs[:, 0]                         # per-batch BCE sums
    ssqs = outs[:, 1]                             # per-batch ||s||^2
    bce = np.mean(-bce_sums / N)
    feat = 1.0 - np.sum(ssqs) / (B * float(N) * float(N))
    return np.asarray(bce + feat, dtype=np.float32)
